# revision 72
# baseline (speedup 1.0000x reference)
"""Trainium2 Bass kernel for nn_CrossAttention (linear/efficient attention).

Math: out = x + bo + x_flat @ W_attn where
  W_attn = sum_h Wq_h @ cm_h @ Wo_h,
  cm_h  = softmax_n(k_h)^T @ v_h,  k = ctx_flat @ Wk, v = ctx_flat @ Wv.
(The q projection folds into W_attn.)

Sharding: 8 cores = 4 batches x 2 token-halves. Each core computes partial
[num|den] softmax statistics over its 8192 tokens; a pairwise AllReduce
merges them; each core then applies W_attn to its own token half.

Dataflow: all big matmuls run fp8 DoubleRow (256-deep contraction per
pass). ctx/x arrive fp8 (+x fp16 for the residual) from the host; the
output is stored fp16 and widened on the host. The residual x + bo is
precomputed into the output buffers by ACT/DVE during the AllReduce
window, phase 2 then accumulates W_attn^T x on top. cm-stat matmuls are
software-pipelined one token-pair behind the projections so the PE never
stalls on exp/copy latency.

Timing structure (per perfetto traces on these cores):
- Each dma_start costs ~640ns of serial Sync-engine issue time; wkv8 is
  issued first so the PE pre-warm (14 matmuls) can ramp the p-state while
  the first ctx chunk is in flight.
- The Tensor engine drops its clock when idle >100ns and needs ~3us of
  continuous work for full speed (259ns per 512-col fp8-DR matmul steady
  vs ~450 post-gap) — hence the pinned warm-matmul blocks.
- The collective channel bringup is ~40us from the dummy AR's trigger
  (itself runtime-bound at ~20us); back-to-back collectives serialize on
  the CC engine with a ~6-8us gap, so a single stats-AllReduce at
  phase-1 end beats a split pair.
- Peer stats arrive at a fixed ~86-88us wall (launch stagger + the
  peer's own dummy-AR gate), which this core covers with warm matmuls +
  residual precompute. Everything after peer arrival is the critical
  path: mesh tail ~8us, W_attn build ~6us, phase 2 ~13us
  (accumulate-bound, DVE 22 / ACT+GPSIMD 10), out-store drain ~10us
  (DMA-throughput-bound, hence half-chunk stores).
"""

import sys

if "/opt/trn_rl_repo" not in sys.path:
    sys.path.insert(0, "/opt/trn_rl_repo")

import numpy as np
import ml_dtypes

B = 4
C = 256          # channels (DIM)
N_FULL = 16384   # tokens per batch (128*128)
T = 8192         # tokens per core
HEADS = 8
DH = 64
INNER = 512
NCORES = 8
CHUNK = 2048
NCH = T // CHUNK      # 4
SUBS = CHUNK // 128   # 16

_CACHE: dict = {}
LAST_RESULTS = None   # BassKernelResults of the most recent run (for profiling)
TRACE = False         # set True before calling kernel() to capture a trace
TRACE_CORES = None    # optional list of core ids to profile (default [0])


def _build_nc():
    import concourse.mybir as mybir
    import concourse.tile as tile
    from concourse import bacc

    f32, f16, f8 = mybir.dt.float32, mybir.dt.float16, mybir.dt.float8e4
    AF = mybir.ActivationFunctionType
    DR = mybir.MatmulPerfMode.DoubleRow
    ADD = mybir.AluOpType.add

    nc = bacc.Bacc("TRN2", target_bir_lowering=False, debug=False)

    xh = nc.dram_tensor("xh", [C, T], f16, kind="ExternalInput")
    x8d = nc.dram_tensor("x8", [C, T], f8, kind="ExternalInput")
    ch = nc.dram_tensor("ch", [C, T], f8, kind="ExternalInput")
    wkv = nc.dram_tensor("wkv", [C, 2 * INNER], f8, kind="ExternalInput")
    wqt = nc.dram_tensor("wqt", [INNER, C], f16, kind="ExternalInput")
    wo = nc.dram_tensor("wo", [INNER, C], f16, kind="ExternalInput")
    bo = nc.dram_tensor("bo", [C, 1], f32, kind="ExternalInput")
    out = nc.dram_tensor("out", [C, T], f16, kind="ExternalOutput")

    xh_r = xh.ap().rearrange("(kc p) n -> p kc n", p=128)
    x8_r = x8d.ap().rearrange("(kc p) n -> p kc n", p=128)
    ch_r = ch.ap().rearrange("(kc p) n -> p kc n", p=128)
    out_r = out.ap().rearrange("(oc p) n -> p oc n", p=128)

    with tile.TileContext(nc) as tc:
        with (
            tc.tile_pool(name="wpool", bufs=1) as wpool,
            tc.tile_pool(name="spool", bufs=3) as spool,
            tc.tile_pool(name="ppool", bufs=4) as ppool,
            tc.tile_pool(name="x16pool", bufs=1) as x16pool,
            tc.tile_pool(name="x8pool", bufs=1) as x8pool,
            tc.tile_pool(name="obuf", bufs=1) as obuf,
            tc.tile_pool(name="dpool", bufs=1, space="DRAM") as dpool,
        ):
            def load_ctx8(ci, nsplit=1):
                ctx8 = spool.tile([128, 2, CHUNK], f8, tag="ctx8", name="ctx8")
                # split the load into several dma_starts so descriptors
                # spread across queues and the data lands sooner
                step = CHUNK // nsplit
                for s in range(nsplit):
                    nc.sync.dma_start(
                        ctx8[:, :, s * step : (s + 1) * step],
                        ch_r[
                            :, :, ci * CHUNK + s * step : ci * CHUNK + (s + 1) * step
                        ],
                    )
                return ctx8

            # wkv8 first: it gates the PE pre-warm (~640ns serial Sync-issue
            # cost per dma_start, so order matters)
            wkv8 = wpool.tile([128, 2, 2 * INNER], f8)
            nc.sync.dma_start(wkv8[:], wkv.ap().rearrange("(kc p) o -> p kc o", p=128))

            # dummy AllReduce: pays the ~40us collective channel bringup while
            # phase 1 runs. Its trigger is runtime-bound (~20us) so the input
            # only has to be there by then.
            bo_sb = wpool.tile([128, 2], f32)
            nc.sync.dma_start(bo_sb[:], bo.ap().rearrange("(oc p) x -> p (oc x)", p=128))
            ccw_in = dpool.tile([128, 4], f32)
            ccw_out = dpool.tile([128, 16], f32)
            nc.sync.dma_start(ccw_in[:, 0:2], bo_sb[:])
            nc.sync.dma_start(ccw_in[:, 2:4], bo_sb[:])
            # dummy on a DIFFERENT replica grouping (2x4) than the real
            # stats-AR (pairs): keeps the real AR off the dummy's channel
            # (separate CC engine, no teardown serialization). AllGather:
            # no reduce stage, so the dummy's mesh — which gates the real
            # AR's trigger on both cores — completes sooner.
            nc.gpsimd.collective_compute(
                "AllGather",
                mybir.AluOpType.bypass,
                replica_groups=[[0, 1, 2, 3], [4, 5, 6, 7]],
                ins=[ccw_in.opt()],
                outs=[ccw_out.opt()],
            )

            ctx8_next = load_ctx8(0, nsplit=2)

            wqt16 = wpool.tile([128, 4, C], f16)
            nc.sync.dma_start(
                wqt16[:], wqt.ap().rearrange("(hc p) i -> p hc i", p=128)
            )
            # wo packed per head-PAIR: partitions 0:64 = head 2hp's rows,
            # 64:128 = head 2hp+1's — enables 128-deep weff contraction
            wo16 = wpool.tile([128, 4, C], f16)
            nc.sync.dma_start(wo16[:], wo.ap().rearrange("(hp p) o -> p hp o", p=128))

            # ---- phase 1: accumulate per-head [num | den] over local tokens ----
            # cm_ps[hp] rows 0:64   = head 2hp   : cols 0:64 num, col 64 den
            #           rows 64:128 = head 2hp+1 : cols 65:129 num, col 129 den
            # Stats are split into two halves (pairs 0:15 / 16:31); the first
            # half's AllReduce is issued mid-phase so its latency (channel +
            # peer skew) hides under the rest of phase 1.
            # NOTE: collectives serialize on the CC engine with ~7us gaps
            # between meshes, so a single stats AllReduce at phase-1 end
            # beats a split pair (measured).
            cm_sbB = wpool.tile([128, 4, 130], f32)
            x16_tiles = []
            x8_tiles = []
            NPAIR = NCH * SUBS // 2

            ccB_in = dpool.tile([128, 4, 65], f32)
            ccB_out = dpool.tile([128, 4, 65], f32)

            with (
                tc.tile_pool(name="ps_cm", bufs=1, space="PSUM") as ps_cm,
                tc.tile_pool(name="ps_kv", bufs=3, space="PSUM") as ps_kv,
            ):
                # PE pre-warm: ramps the Tensor engine to max p-state while
                # the first ctx chunk is still in flight (gates on wkv8 only)
                for _ in range(14):
                    kw = ps_kv.tile([128, INNER], f32, tag="k")
                    nc.tensor.matmul(
                        kw[:],
                        lhsT=wkv8[:, :, 0:128],
                        rhs=wkv8[:, :, 0:INNER],
                        start=True,
                        stop=True,
                        perf_mode=DR,
                    )

                # two hp-groups share a PSUM bank: [128, 260] = 1040 B fits
                # one 2 KiB bank, freeing banks for a deeper ps_kv pool
                cm_ps = [
                    ps_cm.tile([128, 260], f32, tag=f"cm{i}", name=f"cm{i}")
                    for i in range(2)
                ]

                def cm_slot(hp):
                    return cm_ps[hp >> 1][:, (hp & 1) * 130 : (hp & 1) * 130 + 130]

                def emit_cm(pair, idx):
                    kexp8, vcat8 = pair
                    for hp in range(4):
                        nc.tensor.matmul(
                            cm_slot(hp),
                            lhsT=kexp8[:, :, hp * 128 : (hp + 1) * 128],
                            rhs=vcat8[:, :, 2 * hp : 2 * hp + 2, :],
                            start=(idx == 0),
                            stop=(idx == NPAIR - 1),
                            perf_mode=DR,
                        )

                pend = None
                pair_idx = 0
                for ci in range(NCH):
                    ctx8 = ctx8_next
                    if ci + 1 < NCH:
                        ctx8_next = load_ctx8(ci + 1)
                    # phase-2 x tiles: fp8 for the matmul, f16 for the
                    # residual; both straight from HBM, kept resident
                    x16 = x16pool.tile(
                        [128, 2, CHUNK], f16, tag=f"x16_{ci}", name=f"x16_{ci}"
                    )
                    nc.sync.dma_start(
                        x16[:], xh_r[:, :, ci * CHUNK : (ci + 1) * CHUNK]
                    )
                    x16_tiles.append(x16)
                    x8 = x8pool.tile(
                        [128, 2, CHUNK], f8, tag=f"x8_{ci}", name=f"x8_{ci}"
                    )
                    nc.sync.dma_start(
                        x8[:], x8_r[:, :, ci * CHUNK : (ci + 1) * CHUNK]
                    )
                    x8_tiles.append(x8)

                    for s in range(SUBS):
                        tok = slice(s * 128, (s + 1) * 128)
                        half = s % 2
                        if half == 0:
                            kexp8 = ppool.tile([128, 2, INNER], f8, tag="kexp")
                            vcat8 = ppool.tile([128, 2, 8, 65], f8, tag="vcat")
                            nc.gpsimd.memset(vcat8[:, :, :, 64], 1.0)
                            # flush the PREVIOUS pair's cm matmuls now: its
                            # exp/copy ops had a full pair of time to finish,
                            # so the PE never stalls on them
                            if pend is not None:
                                emit_cm(pend, pair_idx - 1)
                        # K / V projections: contraction 256 in one
                        # DoubleRow pass each
                        k_ps = ps_kv.tile([128, INNER], f32, tag="k")
                        nc.tensor.matmul(
                            k_ps[:],
                            lhsT=ctx8[:, :, tok],
                            rhs=wkv8[:, :, 0:INNER],
                            start=True,
                            stop=True,
                            perf_mode=DR,
                        )
                        nc.scalar.activation(
                            kexp8[:, half, :], k_ps[:], AF.Exp
                        )
                        v_ps = ps_kv.tile([128, INNER], f32, tag="v")
                        nc.tensor.matmul(
                            v_ps[:],
                            lhsT=ctx8[:, :, tok],
                            rhs=wkv8[:, :, INNER : 2 * INNER],
                            start=True,
                            stop=True,
                            perf_mode=DR,
                        )
                        nc.vector.tensor_copy(
                            vcat8[:, half, :, 0:64],
                            v_ps[:].rearrange("p (h e) -> p h e", h=8),
                        )
                        if half == 1:
                            pend = (kexp8, vcat8)
                            pair_idx += 1
                emit_cm(pend, NPAIR - 1)
                for hp in range(4):
                    nc.vector.tensor_copy(cm_sbB[:, hp, :], cm_slot(hp))

            # ---- pairwise AllReduce of [num|den] across the 2 token halves ----
            nc.sync.dma_start(ccB_in[0:64, :, :], cm_sbB[0:64, :, 0:65])
            nc.sync.dma_start(ccB_in[64:128, :, :], cm_sbB[64:128, :, 65:130])
            nc.gpsimd.collective_compute(
                "AllReduce",
                mybir.AluOpType.add,
                replica_groups=[[0, 1], [2, 3], [4, 5], [6, 7]],
                ins=[ccB_in.opt()],
                outs=[ccB_out.opt()],
            )
            # fetch the merged stats split across two issue queues so the
            # transfer halves land in parallel
            mm_sb = wpool.tile([128, 4, 65], f32)
            nc.sync.dma_start(mm_sb[0:64, :, :], ccB_out[0:64, :, :])
            nc.scalar.dma_start(mm_sb[64:128, :, :], ccB_out[64:128, :, :])

            # ---- residual precompute: obuf[ci] = x + bo, during the CC
            # window (ACT and DVE are otherwise idle there). bo_gate is
            # bo + 0*cm_sb: the fake cm_sb read pins these ops AFTER
            # phase 1 so the scheduler cannot hoist them into the middle
            # of the exp stream ----
            bo_gate = wpool.tile([128, 2], f32)
            nc.vector.scalar_tensor_tensor(
                bo_gate[:],
                in0=cm_sbB[:, 0, 0:2],
                scalar=0.0,
                in1=bo_sb[:],
                op0=mybir.AluOpType.mult,
                op1=mybir.AluOpType.add,
            )
            out_tiles = []
            for ci in range(NCH):
                ot = obuf.tile([128, 2, CHUNK], f16, tag=f"out_{ci}", name=f"out_{ci}")
                out_tiles.append(ot)
                for oc in range(2):
                    if (ci + oc) % 2 == 0:
                        nc.scalar.activation(
                            ot[:, oc, :],
                            x16_tiles[ci][:, oc, :],
                            AF.Identity,
                            bias=bo_gate[:, oc : oc + 1],
                        )
                    else:
                        nc.vector.tensor_scalar_add(
                            ot[:, oc, :],
                            x16_tiles[ci][:, oc, :],
                            bo_gate[:, oc : oc + 1],
                        )

            # keep the PE (and die) clocked through the AllReduce window.
            # warm16 = wqt16 + 0*cm_sbB: the fake cm_sbB read pins the warm
            # matmuls AFTER phase 1 so the scheduler cannot hoist them into
            # the middle of the proj stream (where they'd add dead PE time).
            warm16 = wpool.tile([128, 128], f16)
            nc.vector.scalar_tensor_tensor(
                warm16[:],
                in0=cm_sbB[:, 0, 0:128],
                scalar=0.0,
                in1=wqt16[:, 0, 0:128],
                op0=mybir.AluOpType.mult,
                op1=mybir.AluOpType.add,
            )
            with tc.tile_pool(name="ps_warm", bufs=1, space="PSUM") as ps_warm:
                warm_ps = ps_warm.tile(
                    [128, 2 * C], f32, tag="warm", name="warm_ps"
                )
                for _ in range(68):
                    nc.tensor.matmul(
                        warm_ps[:],
                        lhsT=warm16[:],
                        rhs=wqt16[:, 0:2, :],
                        start=True,
                        stop=True,
                    )


            # ---- normalize cm, build W_attn = sum_h Wq_h cm_h Wo_h (fp8) ----
            deninv = wpool.tile([128, 4], f32)
            # block-diagonal cmn per head pair: rows 0:64 hold head 2hp's
            # normalized cm in cols 0:64, rows 64:128 hold head 2hp+1's in
            # cols 64:128, off-blocks zero. One 128-deep matmul then yields
            # BOTH heads' m1t at once (4 matmuls instead of 8), and weff
            # contracts 128-deep over packed m1t/wo (8 matmuls instead of 16).
            cmn16 = wpool.tile([128, 4, 128], f16)
            m1t16 = wpool.tile([128, 4, C], f16)
            wattn8 = wpool.tile([128, 2, C], f8)
            for hp in range(4):
                nc.gpsimd.memset(cmn16[0:64, hp, 64:128], 0.0)
                nc.gpsimd.memset(cmn16[64:128, hp, 0:64], 0.0)
            with (
                tc.tile_pool(name="ps_m1t", bufs=2, space="PSUM") as ps_m1t,
                tc.tile_pool(name="ps_weff", bufs=1, space="PSUM") as ps_weff,
                tc.tile_pool(name="ps_o", bufs=4, space="PSUM") as ps_o,
            ):
                nc.vector.reciprocal(deninv[:], mm_sb[:, :, 64])
                # normalize into the block-diagonal quadrants, ACT and DVE
                # in parallel to shorten the post-AllReduce serial chain
                for hp in range(4):
                    if hp < 2:
                        nc.scalar.activation(
                            cmn16[0:64, hp, 0:64],
                            mm_sb[0:64, hp, 0:64],
                            AF.Identity,
                            scale=deninv[0:64, hp : hp + 1],
                        )
                        nc.scalar.activation(
                            cmn16[64:128, hp, 64:128],
                            mm_sb[64:128, hp, 0:64],
                            AF.Identity,
                            scale=deninv[64:128, hp : hp + 1],
                        )
                    else:
                        nc.vector.tensor_scalar_mul(
                            cmn16[0:64, hp, 0:64],
                            mm_sb[0:64, hp, 0:64],
                            deninv[0:64, hp : hp + 1],
                        )
                        nc.vector.tensor_scalar_mul(
                            cmn16[64:128, hp, 64:128],
                            mm_sb[64:128, hp, 0:64],
                            deninv[64:128, hp : hp + 1],
                        )
                for hp in range(4):
                    m1t_ps = ps_m1t.tile([128, C], f32, tag="m1t")
                    nc.tensor.matmul(
                        m1t_ps[:],
                        lhsT=cmn16[:, hp, :],
                        rhs=wqt16[:, hp, :],
                        start=True,
                        stop=True,
                    )
                    # alternate the copies across ACT/DVE so they don't pace
                    # the weff stream
                    if hp % 2 == 0:
                        nc.scalar.copy(m1t16[:, hp, :], m1t_ps[:])
                    else:
                        nc.vector.tensor_copy(m1t16[:, hp, :], m1t_ps[:])
                # hp-major weff with both ic accumulators live: both halves of
                # W_attn finish (and cast) together — every phase-2 matmul
                # contracts over both ic planes, so the later cast gates it
                weff_ps0 = ps_weff.tile([128, C], f32, tag="weff0")
                weff_ps1 = ps_weff.tile([128, C], f32, tag="weff1")
                for hp in range(4):
                    nc.tensor.matmul(
                        weff_ps0[:],
                        lhsT=m1t16[:, hp, 0:128],
                        rhs=wo16[:, hp, :],
                        start=(hp == 0),
                        stop=(hp == 3),
                    )
                    nc.tensor.matmul(
                        weff_ps1[:],
                        lhsT=m1t16[:, hp, 128:256],
                        rhs=wo16[:, hp, :],
                        start=(hp == 0),
                        stop=(hp == 3),
                    )
                nc.vector.tensor_copy(wattn8[:, 0, :], weff_ps0[:])
                nc.scalar.copy(wattn8[:, 1, :], weff_ps1[:])

                # ---- phase 2: obuf += W_attn^T x (fp8 DoubleRow), store ----
                NH = CHUNK // 512
                for ci in range(NCH):
                    x8 = x8_tiles[ci]
                    ot = out_tiles[ci]
                    for oc in range(2):
                        for nh in range(NH):
                            ts_ = slice(nh * 512, (nh + 1) * 512)
                            o_ps = ps_o.tile([128, 512], f32, tag="o")
                            nc.tensor.matmul(
                                o_ps[:],
                                lhsT=wattn8[:, :, oc * 128 : (oc + 1) * 128],
                                rhs=x8[:, :, ts_],
                                start=True,
                                stop=True,
                                perf_mode=DR,
                            )
                            # accumulate onto the precomputed residual;
                            # DVE takes ~22 blocks incl. the final ones (it is
                            # the fastest path, shortening the store drain),
                            # ACT+GPSIMD take ~10 early/mid blocks
                            blk = ((ci * 2) + oc) * NH + nh
                            if blk >= 22 or blk % 10 < 6:
                                nc.vector.tensor_tensor(
                                    ot[:, oc, ts_], o_ps[:], ot[:, oc, ts_], ADD
                                )
                            else:
                                tmp16 = spool.tile([128, 512], f16, tag="tmp")
                                nc.scalar.copy(tmp16[:], o_ps[:])
                                nc.gpsimd.tensor_add(
                                    ot[:, oc, ts_], tmp16[:], ot[:, oc, ts_]
                                )
                            # half-chunk stores, alternating between the Sync
                            # and Scalar issue queues: a dma_start costs
                            # ~0.7us of serial issue time per queue, and 16
                            # stores on one queue (11us) would outlast the
                            # accumulates
                            if nh == 1:
                                nc.sync.dma_start(
                                    out_r[
                                        :, oc, ci * CHUNK : ci * CHUNK + 1024
                                    ],
                                    ot[:, oc, 0:1024],
                                )
                            elif nh == 3:
                                nc.scalar.dma_start(
                                    out_r[
                                        :,
                                        oc,
                                        ci * CHUNK + 1024 : (ci + 1) * CHUNK,
                                    ],
                                    ot[:, oc, 1024:2048],
                                )

    nc.compile()
    return nc


def _get_nc():
    if "nc" not in _CACHE:
        _CACHE["nc"] = _build_nc()
    return _CACHE["nc"]


def kernel(**inputs) -> np.ndarray:
    global LAST_RESULTS
    from concourse.bass_utils import run_bass_kernel_spmd

    f8 = ml_dtypes.float8_e4m3
    x = np.ascontiguousarray(np.asarray(inputs["x"], dtype=np.float32))
    ctx = np.ascontiguousarray(np.asarray(inputs["context"], dtype=np.float32))
    Wq = np.asarray(inputs["Wq"], dtype=np.float32)
    Wk = np.asarray(inputs["Wk"], dtype=np.float32)
    Wv = np.asarray(inputs["Wv"], dtype=np.float32)
    Wo = np.asarray(inputs["Wo"], dtype=np.float32)
    bo = np.ascontiguousarray(
        np.asarray(inputs["bo"], dtype=np.float32).reshape(C, 1)
    )
    wkv8 = np.ascontiguousarray(
        np.concatenate([Wk, Wv], axis=1).astype(f8)
    )
    wqt16 = np.ascontiguousarray(Wq.T.astype(np.float16))
    wo16 = np.ascontiguousarray(Wo.astype(np.float16))

    x16 = x.reshape(B, C, N_FULL).astype(np.float16)
    x8f = x.reshape(B, C, N_FULL).astype(f8)
    c8 = ctx.reshape(B, C, N_FULL).astype(f8)

    in_maps = []
    for c in range(NCORES):
        b, t = c // 2, c % 2
        sl = slice(t * T, (t + 1) * T)
        in_maps.append(
            {
                "xh": np.ascontiguousarray(x16[b, :, sl]),
                "x8": np.ascontiguousarray(x8f[b, :, sl]),
                "ch": np.ascontiguousarray(c8[b, :, sl]),
                "wkv": wkv8,
                "wqt": wqt16,
                "wo": wo16,
                "bo": bo,
            }
        )

    nc = _get_nc()
    kw = {}
    if TRACE and TRACE_CORES is not None:
        kw["trace_cores"] = TRACE_CORES
    res = run_bass_kernel_spmd(nc, in_maps, list(range(NCORES)), trace=TRACE, **kw)
    LAST_RESULTS = res

    out = np.empty((B, C, N_FULL), dtype=np.float32)
    for c in range(NCORES):
        b, t = c // 2, c % 2
        out[b, :, t * T : (t + 1) * T] = res.results[c]["out"].astype(np.float32)
    return out.reshape(B, C, 128, 128)

